# revision 18
# baseline (speedup 1.0000x reference)
"""Multi-head attention, tensor-parallel over heads on 8 Trainium2 NeuronCores.

Contract: kernel(**inputs) takes the FULL unsharded inputs from
reference.setup_inputs() and returns the FULL [2, 2048, 1024] fp32 output.

Sharding: 16 heads / 8 cores = 2 heads per core (tensor parallel).
Each core receives the full (host-transposed, bf16-cast) activations and its
2-head slice of wq/wk/wv plus the matching wo columns; it computes
  qhT/khT = (x @ Wq_c^T)^T  (head-dim on partitions)
  vh      =  x @ Wv_c^T      (seq on partitions)
  S^T     = khT^T.T @ qhT    (flash-style, k-seq on PSUM partitions)
  expS    = exp(S^T / 8)     (no max subtraction: logits ~ N(0,1))
  att^T   = [vh | 1]^T-accumulated AV matmul (row 64 = softmax denominator)
  att_n   = att^T * recip(denom)  (broadcast along partitions)
  partial = att_n^T @ wo_c^T
The host sums the 8 partials and adds the (linear) bv/bo bias terms.
"""

import sys

import numpy as np

sys.path.insert(0, "/opt/trn_rl_repo")

import ml_dtypes  # noqa: E402

import concourse.bacc as bacc  # noqa: E402
import concourse.mybir as mybir  # noqa: E402
import concourse.tile as tile  # noqa: E402
from concourse.bass_utils import run_bass_kernel_spmd  # noqa: E402

D_MODEL = 1024
NUM_HEADS = 16
DEPTH = 64
B, S = 2, 2048
BS = B * S  # 4096
NCORES = 8
HPC = NUM_HEADS // NCORES  # 2 heads per core
HD = HPC * DEPTH  # 128 head dims per core
KC = D_MODEL // 128  # 8 contraction chunks of 128
NT = 512  # moving-operand (free dim) tile
QT = S // NT  # 4 q tiles per batch
KT = S // 128  # 16 k tiles per batch
P = 128

FP32 = mybir.dt.float32
BF16 = mybir.dt.bfloat16
NPBF = ml_dtypes.bfloat16

# DVE rejects partition-step-0 APs ("partition dimension must have nonzero
# step"), so the recip row is materialized via gpsimd.partition_broadcast.
BCAST_VIA_GPSIMD = True


def _build_program(loop_iters=1):
    nc = bacc.Bacc(
        "TRN2", target_bir_lowering=False, debug=False, num_devices=NCORES
    )
    io = {}
    io["xqT"] = nc.dram_tensor("xqT", [D_MODEL, BS], BF16, kind="ExternalInput").ap()
    io["xkT"] = nc.dram_tensor("xkT", [D_MODEL, BS], BF16, kind="ExternalInput").ap()
    io["xvT"] = nc.dram_tensor("xvT", [D_MODEL, BS], BF16, kind="ExternalInput").ap()
    io["wqT"] = nc.dram_tensor("wqT", [D_MODEL, HD], BF16, kind="ExternalInput").ap()
    io["wkT"] = nc.dram_tensor("wkT", [D_MODEL, HD], BF16, kind="ExternalInput").ap()
    io["wvT"] = nc.dram_tensor("wvT", [D_MODEL, HD], BF16, kind="ExternalInput").ap()
    io["woT"] = nc.dram_tensor("woT", [HD, D_MODEL], BF16, kind="ExternalInput").ap()
    io["bq"] = nc.dram_tensor("bq", [HD, 1], FP32, kind="ExternalInput").ap()
    io["bk"] = nc.dram_tensor("bk", [HD, 1], FP32, kind="ExternalInput").ap()
    io["ident"] = nc.dram_tensor("ident", [P, P], BF16, kind="ExternalInput").ap()
    io["out"] = nc.dram_tensor("out", [BS, D_MODEL], FP32, kind="ExternalOutput").ap()

    with tile.TileContext(nc, trace_sim=False) as tc:
        if loop_iters > 1:
            with tc.For_i(0, loop_iters, 1):
                _emit(tc, nc, io)
        else:
            _emit(tc, nc, io)
    nc.compile()
    return nc


def _emit(tc, nc, io):
    EXP = mybir.ActivationFunctionType.Exp
    with (
        tc.tile_pool(name="const", bufs=1) as cpool,
        tc.tile_pool(name="acts", bufs=1) as apool,
        tc.tile_pool(name="xin", bufs=4) as xpool,
        tc.tile_pool(name="es", bufs=6) as spool,
        tc.tile_pool(name="rc", bufs=4) as rpool,
        tc.tile_pool(name="ot", bufs=3) as opool,
        tc.tile_pool(name="pp", bufs=2, space="PSUM") as pp,
        tc.tile_pool(name="ps", bufs=2, space="PSUM") as ps,
        tc.tile_pool(name="pav", bufs=2, space="PSUM") as pavp,
    ):
        po = pp  # outproj + transpose psum share the projection slots
        # --- constants ---
        wq_sb = cpool.tile([P, KC, HD], BF16, tag="wq")
        wk_sb = cpool.tile([P, KC, HD], BF16, tag="wk")
        wv_sb = cpool.tile([P, KC, HD], BF16, tag="wv")
        nc.sync.dma_start(wq_sb, io["wqT"].rearrange("(kc p) m -> p kc m", p=P))
        nc.sync.dma_start(wk_sb, io["wkT"].rearrange("(kc p) m -> p kc m", p=P))
        nc.sync.dma_start(wv_sb, io["wvT"].rearrange("(kc p) m -> p kc m", p=P))
        wo_sb = cpool.tile([P, D_MODEL], BF16, tag="wo")
        nc.sync.dma_start(wo_sb, io["woT"])
        bq_sb = cpool.tile([P, 1], FP32, tag="bq")
        bk_sb = cpool.tile([P, 1], FP32, tag="bk")
        nc.sync.dma_start(bq_sb, io["bq"])
        nc.sync.dma_start(bk_sb, io["bk"])
        id_sb = cpool.tile([P, P], BF16, tag="ident")
        nc.sync.dma_start(id_sb, io["ident"])

        # --- persistent activations (split per batch so b=1 proj overlaps b=0 attn) ---
        qhT = [apool.tile([P, S], BF16, tag=f"qhT{b}", name=f"qhT{b}") for b in range(B)]
        khT = [apool.tile([P, S], BF16, tag=f"khT{b}", name=f"khT{b}") for b in range(B)]
        att = [apool.tile([P, S], BF16, tag=f"att{b}", name=f"att{b}") for b in range(B)]
        vhT = [apool.tile([P, S], BF16, tag=f"vhT{b}", name=f"vhT{b}") for b in range(B)]
        # vh: [part=seq%128, seq-tile slot, head, 64 depth + ones col + pad]
        vh_sb = apool.tile([P, BS // P, HPC, 66], BF16, tag="vh")
        nc.vector.memset(vh_sb[:, :, :, 64:66], 1.0)

        # --- projections (transposed layout; one merged DMA per (tensor, nt)) ---
        for b in range(B):
            for nt in range(QT):
                g0 = b * S + nt * NT
                lc = slice(nt * NT, (nt + 1) * NT)
                for name, w_sb, b_sb, dst in (
                    ("xqT", wq_sb, bq_sb, qhT[b]),
                    ("xkT", wk_sb, bk_sb, khT[b]),
                    ("xvT", wv_sb, None, vhT[b]),
                ):
                    xt = xpool.tile([P, KC, NT], BF16, tag="xt")
                    nc.sync.dma_start(
                        xt,
                        io[name][:, g0 : g0 + NT].rearrange(
                            "(kc p) n -> p kc n", p=P
                        ),
                    )
                    psq = pp.tile([P, NT], FP32, tag="pp")
                    for kc in range(KC):
                        nc.tensor.matmul(
                            psq, lhsT=w_sb[:, kc], rhs=xt[:, kc],
                            start=(kc == 0), stop=(kc == KC - 1),
                        )
                    if b_sb is None:
                        nc.vector.tensor_copy(dst[:, lc], psq)
                    else:
                        nc.vector.tensor_scalar_add(dst[:, lc], psq, b_sb)
            # transpose vhT -> vh (natural k-seq-on-partitions layout for AV)
            for mtl in range(S // P):
                mt = b * (S // P) + mtl
                pst = po.tile([P, P], BF16, tag="pp")
                nc.tensor.transpose(
                    pst, vhT[b][:, mtl * P : (mtl + 1) * P], id_sb
                )
                nc.vector.tensor_copy(vh_sb[:, mt, 0, 0:DEPTH], pst[:, 0:DEPTH])
                nc.vector.tensor_copy(vh_sb[:, mt, 1, 0:DEPTH], pst[:, DEPTH:HD])

        # --- attention + output projection ---
        for b in range(B):
            for qt in range(QT):
                qc = slice(qt * NT, (qt + 1) * NT)
                # Both heads interleaved: their scores matmuls sit at PE row
                # groups 0-63 / 64-127 (auto tile_position), so adjacent
                # instructions can overlap on distinct sub-arrays.
                pavs = [
                    pavp.tile([DEPTH + 1, NT], FP32, tag="pav", name=f"pav{h}")
                    for h in range(HPC)
                ]
                for kt2 in range(KT // 2):
                    # two k-tiles share one wide PSUM + one exp op per head
                    esl = []
                    for h in range(HPC):
                        hs = slice(h * DEPTH, (h + 1) * DEPTH)
                        pss = ps.tile([P, 2 * NT], FP32, tag="ps", name=f"ps{h}")
                        es = spool.tile([P, 2 * NT], BF16, tag="es", name=f"es{h}")
                        esl.append(es)
                        for j in range(2):
                            kt = 2 * kt2 + j
                            nc.tensor.matmul(
                                pss[:, j * NT : (j + 1) * NT],
                                lhsT=khT[b][hs, kt * P : (kt + 1) * P],
                                rhs=qhT[b][hs, qc],
                                start=True, stop=True,
                                skip_group_check=True,
                            )
                        nc.scalar.activation(es, pss, EXP, scale=0.125)
                    for h in range(HPC):
                        for j in range(2):
                            kt = 2 * kt2 + j
                            nc.tensor.matmul(
                                pavs[h],
                                lhsT=vh_sb[:, b * KT + kt, h, 0 : DEPTH + 1],
                                rhs=esl[h][:, j * NT : (j + 1) * NT],
                                start=(kt == 0), stop=(kt == KT - 1),
                            )
                for h in range(HPC):
                    hs = slice(h * DEPTH, (h + 1) * DEPTH)
                    rc = rpool.tile([1, NT], FP32, tag="rc")
                    nc.vector.reciprocal(rc, pavs[h][DEPTH : DEPTH + 1, :])
                    if BCAST_VIA_GPSIMD:
                        rb = rpool.tile([DEPTH, NT], FP32, tag="rb")
                        nc.gpsimd.partition_broadcast(rb, rc)
                        nc.vector.tensor_mul(att[b][hs, qc], pavs[h][0:DEPTH, :], rb)
                    else:
                        nc.vector.tensor_mul(
                            att[b][hs, qc],
                            pavs[h][0:DEPTH, :],
                            rc.broadcast_to([DEPTH, NT]),
                        )
            for qs in range(S // P):
                r0 = b * S + qs * P
                ot = opool.tile([P, D_MODEL], FP32, tag="ot")
                for mt2 in range(D_MODEL // NT):
                    mc = slice(mt2 * NT, (mt2 + 1) * NT)
                    pso = po.tile([P, NT], FP32, tag="pp")
                    nc.tensor.matmul(
                        pso,
                        lhsT=att[b][:, qs * P : (qs + 1) * P],
                        rhs=wo_sb[:, mc],
                        start=True, stop=True,
                    )
                    nc.vector.tensor_copy(ot[:, mc], pso)
                nc.sync.dma_start(io["out"][r0 : r0 + P, :], ot)


_NC_CACHE = None


def get_nc():
    global _NC_CACHE
    if _NC_CACHE is None:
        _NC_CACHE = _build_program()
    return _NC_CACHE


def make_in_maps(q, k, v, wq, bq, wk, bk, wv, bv, wo, bo):
    """Host-side shard/transpose/cast prep. Returns (in_maps, host_bias_row)."""
    f32 = np.float32
    qT = np.ascontiguousarray(np.asarray(q, f32).reshape(BS, D_MODEL).T).astype(NPBF)
    kT = np.ascontiguousarray(np.asarray(k, f32).reshape(BS, D_MODEL).T).astype(NPBF)
    vT = np.ascontiguousarray(np.asarray(v, f32).reshape(BS, D_MODEL).T).astype(NPBF)
    wq, wk, wv, wo = (np.asarray(a, f32) for a in (wq, wk, wv, wo))
    bq, bk, bv, bo = (np.asarray(a, f32) for a in (bq, bk, bv, bo))
    in_maps = []
    for c in range(NCORES):
        sl = slice(c * HD, (c + 1) * HD)
        in_maps.append(
            {
                "xqT": qT,
                "xkT": kT,
                "xvT": vT,
                "wqT": np.ascontiguousarray(wq[sl, :].T).astype(NPBF),
                "wkT": np.ascontiguousarray(wk[sl, :].T).astype(NPBF),
                "wvT": np.ascontiguousarray(wv[sl, :].T).astype(NPBF),
                "woT": np.ascontiguousarray(wo[:, sl].T).astype(NPBF),
                "bq": np.ascontiguousarray(bq[sl]).reshape(HD, 1),
                "bk": np.ascontiguousarray(bk[sl]).reshape(HD, 1),
                "ident": np.eye(P, dtype=NPBF),
            }
        )
    # bv enters linearly (softmax rows sum to 1): out += bv @ wo.T + bo
    host_bias = (bv @ wo.T + bo).astype(f32)
    return in_maps, host_bias


def run_on_hw(inputs, trace=False, **kw):
    nc = get_nc()
    in_maps, host_bias = make_in_maps(**inputs)
    res = run_bass_kernel_spmd(
        nc, in_maps, list(range(NCORES)), trace=trace, **kw
    )
    acc = np.zeros((BS, D_MODEL), np.float32)
    for c in range(NCORES):
        acc += np.asarray(res.results[c]["out"], np.float32)
    acc += host_bias[None, :]
    return acc.reshape(B, S, D_MODEL), res


def kernel(**inputs):
    out, _ = run_on_hw(inputs, trace=False)
    return out
